# revision 11
# baseline (speedup 1.0000x reference)
"""Causal GQA cross-attention kernel for Trainium2, 8-core SPMD (v2).

Problem: q [2, 2048, 16, 128] f32, kv [2, 2048, 2, 8, 128] f32 ->
out [2, 2048, 16, 128] f32; causal mask (Sq == Sk), GQA with 2 q heads
per kv head, softmax scale 1/sqrt(128).

Sharding: 2 batches x 4 kv-head-pairs -> 8 cores. Each core gets 4 q
heads + 2 kv heads (its GQA groups); no collectives.

v2 changes vs v1:
  - Host pre-transposes Q/K to [head, D, S] and V to [head, S, D] in
    bf16 (ml_dtypes), so the device does single big DMA loads: no
    on-device DMA transposes, no f32->bf16 cast copies.
  - All input loads prefetched upfront (SBUF is big enough).
  - Software-pipelined emission: PV matmuls of pair p-1 are emitted
    after QK+exp of pair p, so the PE queue never waits on ACT.
  - QK matmuls on diagonal pairs trimmed to the causally valid q range
    (the trimmed regions are never read downstream).
  - PV PSUM tiles packed [128, 2, 129] (two q blocks per bank), bufs=4
    so consecutive superblocks overlap; output stored p-major for
    single-DMA stores.
  - Softmax denominator via ones-augmented V column (col 128); host
    divides and transposes back.
"""

import math
import os
from collections import deque

import ml_dtypes
import numpy as np

import sys

sys.path.insert(0, "/opt/trn_rl_repo")

import concourse.bass as bass  # noqa: E402
import concourse.mybir as mybir  # noqa: E402
import concourse.tile as tile  # noqa: E402
from concourse import bacc  # noqa: E402
from concourse.bass_utils import run_bass_kernel_spmd  # noqa: E402

B, SQ, SK, H, HKV, D = 2, 2048, 2048, 16, 8, 128
NCORES = 8
NQH = H * B // NCORES  # 4 q heads per core
NKVH = HKV * B // NCORES  # 2 kv heads per core
P = 128
NQB = SQ // P  # 16 q blocks of 128
NSB = 4  # q superblocks of 512
SBW = 512
NKB = SK // P  # 16 k blocks
SCALE = 1.0 / math.sqrt(D)

F32 = mybir.dt.float32
BF16 = mybir.dt.bfloat16

LAST_RESULTS = None
_CACHE = {}


def build_module():
    nc = bacc.Bacc(None, target_bir_lowering=False)

    q_d = nc.dram_tensor("q", [NQH, D, SQ], BF16, kind="ExternalInput")
    k_d = nc.dram_tensor("k", [NKVH, D, SK], BF16, kind="ExternalInput")
    v_d = nc.dram_tensor("v", [NKVH, SK, D], BF16, kind="ExternalInput")
    # p-major output so [128, nq, 129] SBUF tiles store with one DMA
    o_d = nc.dram_tensor("o", [NQH, P, NQB, D + 1], F32, kind="ExternalOutput")

    with tile.TileContext(nc) as tc:
        with (
            tc.tile_pool(name="const", bufs=1) as constp,
            tc.tile_pool(name="kt", bufs=2) as ktp,
            tc.tile_pool(name="qt", bufs=4) as qtp,
            tc.tile_pool(name="vaug", bufs=2) as vap,
            tc.tile_pool(name="pt", bufs=6) as ptp,
            tc.tile_pool(name="outs", bufs=4) as outp,
            tc.tile_pool(name="pst", bufs=2, space="PSUM") as pstp,
            tc.tile_pool(name="ppv", bufs=4, space="PSUM") as ppvp,
        ):
            # pair masks: m[k, half*512 + q] = 1.0 where
            # (q - k - 128*(r0 + half)) >= 0 else 0.0
            pair_masks = {}
            for r0 in (0, 2):
                m = constp.tile([P, 2 * SBW], BF16, tag=f"mask{r0}")
                nc.gpsimd.memset(m[:], 1.0)
                nc.gpsimd.affine_select(
                    out=m[:].rearrange("p (h q) -> p h q", h=2),
                    in_=m[:].rearrange("p (h q) -> p h q", h=2),
                    compare_op=mybir.AluOpType.is_ge,
                    fill=0.0,
                    base=-P * r0,
                    pattern=[[-P, 2], [1, SBW]],
                    channel_multiplier=-1,
                )
                pair_masks[r0] = m

            # ---- prefetch all inputs across three DMA rings ----
            # SP, ACT, and gpsimd each own a DMA ring; spreading the loads
            # parallelizes ring init + transfer. ACT issues cost nothing
            # before the first exp (it idles waiting on QK anyway).
            kts, vaugs, qts = {}, {}, {}
            for g in range(NKVH):
                kts[g] = ktp.tile([P, SK], BF16, tag="kt", name=f"kt{g}")
                vaugs[g] = vap.tile([P, NKB, D + 1], BF16, tag="vaug", name=f"va{g}")
            for h in range(NQH):
                qts[h] = qtp.tile([P, SQ], BF16, tag="qt", name=f"qt{h}")

            # Critical path (needed in the first ~15us) on the SP HWDGE ring;
            # everything else on the gpsimd SWDGE ring (starts later, but
            # those tiles aren't needed until ~25us+). ACT issues no DMAs so
            # the exp stream starts as soon as the first QK lands.
            nc.gpsimd.memset(vaugs[0][:, :, D : D + 1], 1.0)
            nc.gpsimd.memset(vaugs[1][:, :, D : D + 1], 1.0)
            nc.sync.dma_start(kts[0][:, 0 : 2 * SBW], k_d[0][:, 0 : 2 * SBW])
            nc.sync.dma_start(qts[0][:, 0:SBW], q_d[0][:, 0:SBW])
            nc.sync.dma_start(kts[0][:, 2 * SBW : SK], k_d[0][:, 2 * SBW : SK])
            nc.sync.dma_start(qts[0][:, SBW:SQ], q_d[0][:, SBW:SQ])
            nc.sync.dma_start(
                vaugs[0][:, :, 0:D], v_d[0].rearrange("(kb p) d -> p kb d", p=P)
            )
            nc.sync.dma_start(kts[1][:], k_d[1])
            nc.gpsimd.dma_start(qts[1][:], q_d[1])
            nc.gpsimd.dma_start(
                vaugs[1][:, :, 0:D], v_d[1].rearrange("(kb p) d -> p kb d", p=P)
            )
            nc.gpsimd.dma_start(qts[2][:], q_d[2])
            nc.gpsimd.dma_start(qts[3][:], q_d[3])

            # ---- software-pipelined attention stream ----
            # QK q-range start (in units of 128 cols) for block kb within
            # superblock sb: fully-masked supradiagonal regions trimmed.
            def q_lo(sb, kb):
                return max(0, kb - 4 * sb)

            pending = deque()
            pvmap = {}

            def drain():
                h, sb, pair, pt = pending.popleft()
                g = h // 2
                key = (h, sb)
                if key not in pvmap:
                    pvmap[key] = [
                        ppvp.tile(
                            [P, 2, D + 1], F32, tag="ppv", name=f"pv{h}_{sb}_{i}"
                        )
                        for i in range(2)
                    ]
                pvs = pvmap[key]
                for half in (0, 1):
                    kb = 2 * pair + half
                    for jj in range(4):
                        qb = 4 * sb + jj
                        if kb > qb:
                            continue
                        # start=True clears has_written bits for the WHOLE
                        # bank, so only the first region (even jj) of each
                        # bank may use it; the odd-jj group's first matmul
                        # relies on overwrite-where-bit-clear semantics.
                        nc.tensor.matmul(
                            pvs[jj // 2][:, jj % 2, :],
                            pt[:, half * SBW + jj * P : half * SBW + (jj + 1) * P],
                            vaugs[g][:, kb, :],
                            start=(kb == 0 and jj % 2 == 0),
                            stop=(kb == qb),
                            skip_group_check=(jj % 2 == 1),
                        )
                if pair == 2 * sb + 1:  # last pair of this superblock
                    for i in range(2):
                        ot = outp.tile(
                            [P, 2, D + 1], F32, tag="outs", name=f"o{h}_{sb}_{i}"
                        )
                        nc.vector.tensor_copy(ot[:], pvs[i][:])
                        nc.sync.dma_start(
                            o_d[h][:, 4 * sb + 2 * i : 4 * sb + 2 * i + 2, :], ot[:]
                        )
                    del pvmap[key]

            for h in range(NQH):
                g = h // 2
                qt, kt_g = qts[h], kts[g]
                for sb in range(NSB):
                    for pair in range(2 * sb + 2):
                        st = pstp.tile([P, 2 * SBW], F32, tag="pst", name="st")
                        for half in (0, 1):
                            kb = 2 * pair + half
                            lo = q_lo(sb, kb)
                            nc.tensor.matmul(
                                st[:, half * SBW + lo * P : (half + 1) * SBW],
                                kt_g[:, kb * P : (kb + 1) * P],
                                qt[:, sb * SBW + lo * P : (sb + 1) * SBW],
                                start=True,
                                stop=True,
                            )
                        pt = ptp.tile([P, 2 * SBW], BF16, tag="pt", name="pt")
                        # On the second diagonal pair, cols [0:256) of half 0
                        # are fully masked (kb=4sb+2 vs qb=4sb..): skip them.
                        elo = 2 * P if pair == 2 * sb + 1 else 0
                        nc.scalar.activation(
                            pt[:, elo : 2 * SBW],
                            st[:, elo : 2 * SBW],
                            mybir.ActivationFunctionType.Exp,
                            scale=SCALE,
                        )
                        if pair == 2 * sb:
                            mk = pair_masks[0]
                        elif pair == 2 * sb + 1:
                            mk = pair_masks[2]
                        else:
                            mk = None
                        if mk is not None:
                            nc.vector.tensor_tensor(
                                out=pt[:],
                                in0=pt[:],
                                in1=mk[:],
                                op=mybir.AluOpType.mult,
                            )
                        pending.append((h, sb, pair, pt))
                        if len(pending) > 1:
                            drain()
            while pending:
                drain()

    nc.finalize()
    return nc


def _get_module():
    if "nc" not in _CACHE:
        _CACHE["nc"] = build_module()
    return _CACHE["nc"]


def kernel(q, kv):
    global LAST_RESULTS
    q = np.asarray(q, dtype=np.float32)
    kv = np.asarray(kv, dtype=np.float32)

    nc = _get_module()
    bf = ml_dtypes.bfloat16
    in_maps = []
    for c in range(NCORES):
        b, j = divmod(c, 4)
        # q: [Sq, 4, D] -> [4, D, Sq]
        q_s = np.ascontiguousarray(
            q[b][:, 4 * j : 4 * j + 4, :].transpose(1, 2, 0).astype(bf)
        )
        # k: [Sk, 2, D] -> [2, D, Sk]
        k_s = np.ascontiguousarray(
            kv[b][:, 0, 2 * j : 2 * j + 2, :].transpose(1, 2, 0).astype(bf)
        )
        # v: [Sk, 2, D] -> [2, Sk, D]
        v_s = np.ascontiguousarray(
            kv[b][:, 1, 2 * j : 2 * j + 2, :].transpose(1, 0, 2).astype(bf)
        )
        in_maps.append({"q": q_s, "k": k_s, "v": v_s})

    trace = bool(int(os.environ.get("KERNEL_TRACE", "0")))
    kwargs = {}
    tdir = os.environ.get("KERNEL_TRACE_DIR")
    if tdir:
        kwargs["tmpdir"] = tdir
    if "warm" not in _CACHE:
        # Cold-start device executions intermittently read stale input
        # data (first execution after process start); run one warmup
        # execution and discard it so the measured run is warm.
        run_bass_kernel_spmd(nc, in_maps, core_ids=list(range(NCORES)), trace=False)
        _CACHE["warm"] = True
    res = run_bass_kernel_spmd(
        nc, in_maps, core_ids=list(range(NCORES)), trace=trace, **kwargs
    )
    LAST_RESULTS = res

    out = np.empty((B, SQ, H, D), np.float32)
    for c in range(NCORES):
        b, j = divmod(c, 4)
        o = res.results[c]["o"]  # [NQH, P, NQB, D+1]
        o = o.transpose(0, 2, 1, 3).reshape(NQH, SQ, D + 1)
        norm = o[..., :D] / o[..., D : D + 1]
        out[b, :, 4 * j : 4 * j + 4, :] = np.transpose(norm, (1, 0, 2))
    return out


# revision 12
# speedup vs baseline: 1.0503x; 1.0503x over previous
"""Causal GQA cross-attention kernel for Trainium2, 8-core SPMD (v2).

Problem: q [2, 2048, 16, 128] f32, kv [2, 2048, 2, 8, 128] f32 ->
out [2, 2048, 16, 128] f32; causal mask (Sq == Sk), GQA with 2 q heads
per kv head, softmax scale 1/sqrt(128).

Sharding: 2 batches x 4 kv-head-pairs -> 8 cores. Each core gets 4 q
heads + 2 kv heads (its GQA groups); no collectives.

v2 changes vs v1:
  - Host pre-transposes Q/K to [head, D, S] and V to [head, S, D] in
    bf16 (ml_dtypes), so the device does single big DMA loads: no
    on-device DMA transposes, no f32->bf16 cast copies.
  - All input loads prefetched upfront (SBUF is big enough).
  - Software-pipelined emission: PV matmuls of pair p-1 are emitted
    after QK+exp of pair p, so the PE queue never waits on ACT.
  - QK matmuls on diagonal pairs trimmed to the causally valid q range
    (the trimmed regions are never read downstream).
  - PV PSUM tiles packed [128, 2, 129] (two q blocks per bank), bufs=4
    so consecutive superblocks overlap; output stored p-major for
    single-DMA stores.
  - Softmax denominator via ones-augmented V column (col 128); host
    divides and transposes back.
"""

import math
import os
from collections import deque

import ml_dtypes
import numpy as np

import sys

sys.path.insert(0, "/opt/trn_rl_repo")

import concourse.bass as bass  # noqa: E402
import concourse.mybir as mybir  # noqa: E402
import concourse.tile as tile  # noqa: E402
from concourse import bacc  # noqa: E402
from concourse.bass_utils import run_bass_kernel_spmd  # noqa: E402

B, SQ, SK, H, HKV, D = 2, 2048, 2048, 16, 8, 128
NCORES = 8
NQH = H * B // NCORES  # 4 q heads per core
NKVH = HKV * B // NCORES  # 2 kv heads per core
P = 128
NQB = SQ // P  # 16 q blocks of 128
NSB = 4  # q superblocks of 512
SBW = 512
NKB = SK // P  # 16 k blocks
SCALE = 1.0 / math.sqrt(D)

F32 = mybir.dt.float32
BF16 = mybir.dt.bfloat16

LAST_RESULTS = None
_CACHE = {}


def build_module():
    nc = bacc.Bacc(None, target_bir_lowering=False)

    q_d = nc.dram_tensor("q", [NQH, D, SQ], BF16, kind="ExternalInput")
    k_d = nc.dram_tensor("k", [NKVH, D, SK], BF16, kind="ExternalInput")
    v_d = nc.dram_tensor("v", [NKVH, SK, D], BF16, kind="ExternalInput")
    # p-major output so [128, nq, 129] SBUF tiles store with one DMA
    o_d = nc.dram_tensor("o", [NQH, P, NQB, D + 1], F32, kind="ExternalOutput")

    with tile.TileContext(nc) as tc:
        with (
            tc.tile_pool(name="const", bufs=1) as constp,
            tc.tile_pool(name="kt", bufs=2) as ktp,
            tc.tile_pool(name="qt", bufs=4) as qtp,
            tc.tile_pool(name="vaug", bufs=2) as vap,
            tc.tile_pool(name="pt", bufs=6) as ptp,
            tc.tile_pool(name="outs", bufs=4) as outp,
            tc.tile_pool(name="pst", bufs=2, space="PSUM") as pstp,
            tc.tile_pool(name="ppv", bufs=4, space="PSUM") as ppvp,
        ):
            # pair masks: m[k, half*512 + q] = 1.0 where
            # (q - k - 128*(r0 + half)) >= 0 else 0.0
            pair_masks = {}
            for r0 in (0, 2):
                m = constp.tile([P, 2 * SBW], BF16, tag=f"mask{r0}")
                nc.gpsimd.memset(m[:], 1.0)
                nc.gpsimd.affine_select(
                    out=m[:].rearrange("p (h q) -> p h q", h=2),
                    in_=m[:].rearrange("p (h q) -> p h q", h=2),
                    compare_op=mybir.AluOpType.is_ge,
                    fill=0.0,
                    base=-P * r0,
                    pattern=[[-P, 2], [1, SBW]],
                    channel_multiplier=-1,
                )
                pair_masks[r0] = m

            # ---- prefetch all inputs across three DMA rings ----
            # SP, ACT, and gpsimd each own a DMA ring; spreading the loads
            # parallelizes ring init + transfer. ACT issues cost nothing
            # before the first exp (it idles waiting on QK anyway).
            kts, vaugs, qts = {}, {}, {}
            for g in range(NKVH):
                kts[g] = ktp.tile([P, SK], BF16, tag="kt", name=f"kt{g}")
                vaugs[g] = vap.tile([P, NKB, D + 1], BF16, tag="vaug", name=f"va{g}")
            for h in range(NQH):
                qts[h] = qtp.tile([P, SQ], BF16, tag="qt", name=f"qt{h}")

            # Critical path (needed in the first ~15us) on the SP HWDGE ring;
            # everything else on the gpsimd SWDGE ring (starts later, but
            # those tiles aren't needed until ~25us+). ACT issues no DMAs so
            # the exp stream starts as soon as the first QK lands.
            nc.gpsimd.memset(vaugs[0][:, :, D : D + 1], 1.0)
            nc.gpsimd.memset(vaugs[1][:, :, D : D + 1], 1.0)
            nc.sync.dma_start(kts[0][:], k_d[0])
            nc.sync.dma_start(qts[0][:], q_d[0])
            nc.sync.dma_start(
                vaugs[0][:, :, 0:D], v_d[0].rearrange("(kb p) d -> p kb d", p=P)
            )
            nc.sync.dma_start(kts[1][:], k_d[1])
            nc.gpsimd.dma_start(qts[1][:], q_d[1])
            nc.gpsimd.dma_start(
                vaugs[1][:, :, 0:D], v_d[1].rearrange("(kb p) d -> p kb d", p=P)
            )
            nc.gpsimd.dma_start(qts[2][:], q_d[2])
            nc.gpsimd.dma_start(qts[3][:], q_d[3])

            # ---- software-pipelined attention stream ----
            # QK q-range start (in units of 128 cols) for block kb within
            # superblock sb: fully-masked supradiagonal regions trimmed.
            def q_lo(sb, kb):
                return max(0, kb - 4 * sb)

            pending = deque()
            pvmap = {}

            def drain():
                h, sb, pair, pt = pending.popleft()
                g = h // 2
                key = (h, sb)
                if key not in pvmap:
                    pvmap[key] = [
                        ppvp.tile(
                            [P, 2, D + 1], F32, tag="ppv", name=f"pv{h}_{sb}_{i}"
                        )
                        for i in range(2)
                    ]
                pvs = pvmap[key]
                for half in (0, 1):
                    kb = 2 * pair + half
                    for jj in range(4):
                        qb = 4 * sb + jj
                        if kb > qb:
                            continue
                        # start=True clears has_written bits for the WHOLE
                        # bank, so only the first region (even jj) of each
                        # bank may use it; the odd-jj group's first matmul
                        # relies on overwrite-where-bit-clear semantics.
                        nc.tensor.matmul(
                            pvs[jj // 2][:, jj % 2, :],
                            pt[:, half * SBW + jj * P : half * SBW + (jj + 1) * P],
                            vaugs[g][:, kb, :],
                            start=(kb == 0 and jj % 2 == 0),
                            stop=(kb == qb),
                            skip_group_check=(jj % 2 == 1),
                        )
                if pair == 2 * sb + 1:  # last pair of this superblock
                    for i in range(2):
                        ot = outp.tile(
                            [P, 2, D + 1], F32, tag="outs", name=f"o{h}_{sb}_{i}"
                        )
                        nc.vector.tensor_copy(ot[:], pvs[i][:])
                        nc.sync.dma_start(
                            o_d[h][:, 4 * sb + 2 * i : 4 * sb + 2 * i + 2, :], ot[:]
                        )
                    del pvmap[key]

            for h in range(NQH):
                g = h // 2
                qt, kt_g = qts[h], kts[g]
                for sb in range(NSB):
                    for pair in range(2 * sb + 2):
                        st = pstp.tile([P, 2 * SBW], F32, tag="pst", name="st")
                        for half in (0, 1):
                            kb = 2 * pair + half
                            lo = q_lo(sb, kb)
                            nc.tensor.matmul(
                                st[:, half * SBW + lo * P : (half + 1) * SBW],
                                kt_g[:, kb * P : (kb + 1) * P],
                                qt[:, sb * SBW + lo * P : (sb + 1) * SBW],
                                start=True,
                                stop=True,
                            )
                        pt = ptp.tile([P, 2 * SBW], BF16, tag="pt", name="pt")
                        # On the second diagonal pair, cols [0:256) of half 0
                        # are fully masked (kb=4sb+2 vs qb=4sb..): skip them.
                        elo = 2 * P if pair == 2 * sb + 1 else 0
                        nc.scalar.activation(
                            pt[:, elo : 2 * SBW],
                            st[:, elo : 2 * SBW],
                            mybir.ActivationFunctionType.Exp,
                            scale=SCALE,
                        )
                        if pair == 2 * sb:
                            mk = pair_masks[0]
                        elif pair == 2 * sb + 1:
                            mk = pair_masks[2]
                        else:
                            mk = None
                        if mk is not None:
                            nc.vector.tensor_tensor(
                                out=pt[:],
                                in0=pt[:],
                                in1=mk[:],
                                op=mybir.AluOpType.mult,
                            )
                        pending.append((h, sb, pair, pt))
                        if len(pending) > 1:
                            drain()
            while pending:
                drain()

    nc.finalize()
    return nc


def _get_module():
    if "nc" not in _CACHE:
        _CACHE["nc"] = build_module()
    return _CACHE["nc"]


def kernel(q, kv):
    global LAST_RESULTS
    q = np.asarray(q, dtype=np.float32)
    kv = np.asarray(kv, dtype=np.float32)

    nc = _get_module()
    bf = ml_dtypes.bfloat16
    in_maps = []
    for c in range(NCORES):
        b, j = divmod(c, 4)
        # q: [Sq, 4, D] -> [4, D, Sq]
        q_s = np.ascontiguousarray(
            q[b][:, 4 * j : 4 * j + 4, :].transpose(1, 2, 0).astype(bf)
        )
        # k: [Sk, 2, D] -> [2, D, Sk]
        k_s = np.ascontiguousarray(
            kv[b][:, 0, 2 * j : 2 * j + 2, :].transpose(1, 2, 0).astype(bf)
        )
        # v: [Sk, 2, D] -> [2, Sk, D]
        v_s = np.ascontiguousarray(
            kv[b][:, 1, 2 * j : 2 * j + 2, :].transpose(1, 0, 2).astype(bf)
        )
        in_maps.append({"q": q_s, "k": k_s, "v": v_s})

    trace = bool(int(os.environ.get("KERNEL_TRACE", "0")))
    kwargs = {}
    tdir = os.environ.get("KERNEL_TRACE_DIR")
    if tdir:
        kwargs["tmpdir"] = tdir
    if "warm" not in _CACHE:
        # Cold-start device executions intermittently read stale input
        # data (first execution after process start); run one warmup
        # execution and discard it so the measured run is warm.
        run_bass_kernel_spmd(nc, in_maps, core_ids=list(range(NCORES)), trace=False)
        _CACHE["warm"] = True
    res = run_bass_kernel_spmd(
        nc, in_maps, core_ids=list(range(NCORES)), trace=trace, **kwargs
    )
    LAST_RESULTS = res

    out = np.empty((B, SQ, H, D), np.float32)
    for c in range(NCORES):
        b, j = divmod(c, 4)
        o = res.results[c]["o"]  # [NQH, P, NQB, D+1]
        o = o.transpose(0, 2, 1, 3).reshape(NQH, SQ, D + 1)
        norm = o[..., :D] / o[..., D : D + 1]
        out[b, :, 4 * j : 4 * j + 4, :] = np.transpose(norm, (1, 0, 2))
    return out


# revision 13
# speedup vs baseline: 1.0587x; 1.0080x over previous
"""Causal GQA cross-attention kernel for Trainium2, 8-core SPMD (v2).

Problem: q [2, 2048, 16, 128] f32, kv [2, 2048, 2, 8, 128] f32 ->
out [2, 2048, 16, 128] f32; causal mask (Sq == Sk), GQA with 2 q heads
per kv head, softmax scale 1/sqrt(128).

Sharding: 2 batches x 4 kv-head-pairs -> 8 cores. Each core gets 4 q
heads + 2 kv heads (its GQA groups); no collectives.

v2 changes vs v1:
  - Host pre-transposes Q/K to [head, D, S] and V to [head, S, D] in
    bf16 (ml_dtypes), so the device does single big DMA loads: no
    on-device DMA transposes, no f32->bf16 cast copies.
  - All input loads prefetched upfront (SBUF is big enough).
  - Software-pipelined emission: PV matmuls of pair p-1 are emitted
    after QK+exp of pair p, so the PE queue never waits on ACT.
  - QK matmuls on diagonal pairs trimmed to the causally valid q range
    (the trimmed regions are never read downstream).
  - PV PSUM tiles packed [128, 2, 129] (two q blocks per bank), bufs=4
    so consecutive superblocks overlap; output stored p-major for
    single-DMA stores.
  - Softmax denominator via ones-augmented V column (col 128); host
    divides and transposes back.
"""

import math
import os
from collections import deque

import ml_dtypes
import numpy as np

import sys

sys.path.insert(0, "/opt/trn_rl_repo")

import concourse.bass as bass  # noqa: E402
import concourse.mybir as mybir  # noqa: E402
import concourse.tile as tile  # noqa: E402
from concourse import bacc  # noqa: E402
from concourse.bass_utils import run_bass_kernel_spmd  # noqa: E402

B, SQ, SK, H, HKV, D = 2, 2048, 2048, 16, 8, 128
NCORES = 8
NQH = H * B // NCORES  # 4 q heads per core
NKVH = HKV * B // NCORES  # 2 kv heads per core
P = 128
NQB = SQ // P  # 16 q blocks of 128
NSB = 4  # q superblocks of 512
SBW = 512
NKB = SK // P  # 16 k blocks
SCALE = 1.0 / math.sqrt(D)

F32 = mybir.dt.float32
BF16 = mybir.dt.bfloat16

LAST_RESULTS = None
_CACHE = {}


def build_module():
    nc = bacc.Bacc(None, target_bir_lowering=False)

    q_d = nc.dram_tensor("q", [NQH, D, SQ], BF16, kind="ExternalInput")
    k_d = nc.dram_tensor("k", [NKVH, D, SK], BF16, kind="ExternalInput")
    v_d = nc.dram_tensor("v", [NKVH, SK, D], BF16, kind="ExternalInput")
    # p-major output so [128, nq, 129] SBUF tiles store with one DMA
    o_d = nc.dram_tensor("o", [NQH, P, NQB, D + 1], F32, kind="ExternalOutput")

    with tile.TileContext(nc) as tc:
        with (
            tc.tile_pool(name="const", bufs=1) as constp,
            tc.tile_pool(name="kt", bufs=2) as ktp,
            tc.tile_pool(name="qt", bufs=4) as qtp,
            tc.tile_pool(name="vaug", bufs=2) as vap,
            tc.tile_pool(name="pt", bufs=6) as ptp,
            tc.tile_pool(name="outs", bufs=4) as outp,
            tc.tile_pool(name="pst", bufs=2, space="PSUM") as pstp,
            tc.tile_pool(name="ppv", bufs=4, space="PSUM") as ppvp,
        ):
            # pair masks: m[k, half*512 + q] = 1.0 where
            # (q - k - 128*(r0 + half)) >= 0 else 0.0
            pair_masks = {}
            for r0 in (0, 2):
                m = constp.tile([P, 2 * SBW], BF16, tag=f"mask{r0}")
                nc.gpsimd.memset(m[:], 1.0)
                nc.gpsimd.affine_select(
                    out=m[:].rearrange("p (h q) -> p h q", h=2),
                    in_=m[:].rearrange("p (h q) -> p h q", h=2),
                    compare_op=mybir.AluOpType.is_ge,
                    fill=0.0,
                    base=-P * r0,
                    pattern=[[-P, 2], [1, SBW]],
                    channel_multiplier=-1,
                )
                pair_masks[r0] = m

            # ---- prefetch all inputs across three DMA rings ----
            # SP, ACT, and gpsimd each own a DMA ring; spreading the loads
            # parallelizes ring init + transfer. ACT issues cost nothing
            # before the first exp (it idles waiting on QK anyway).
            kts, vaugs, qts = {}, {}, {}
            for g in range(NKVH):
                kts[g] = ktp.tile([P, SK], BF16, tag="kt", name=f"kt{g}")
                vaugs[g] = vap.tile([P, NKB, D + 1], BF16, tag="vaug", name=f"va{g}")
            for h in range(NQH):
                qts[h] = qtp.tile([P, SQ], BF16, tag="qt", name=f"qt{h}")

            # Critical path (needed in the first ~15us) on the SP HWDGE ring;
            # everything else on the gpsimd SWDGE ring (starts later, but
            # those tiles aren't needed until ~25us+). ACT issues no DMAs so
            # the exp stream starts as soon as the first QK lands.
            nc.gpsimd.memset(vaugs[0][:, :, D : D + 1], 1.0)
            nc.gpsimd.memset(vaugs[1][:, :, D : D + 1], 1.0)
            nc.sync.dma_start(kts[0][:], k_d[0])
            nc.scalar.dma_start(qts[0][:], q_d[0])
            nc.sync.dma_start(
                vaugs[0][:, :, 0:D], v_d[0].rearrange("(kb p) d -> p kb d", p=P)
            )
            nc.sync.dma_start(kts[1][:], k_d[1])
            nc.gpsimd.dma_start(qts[1][:], q_d[1])
            nc.gpsimd.dma_start(
                vaugs[1][:, :, 0:D], v_d[1].rearrange("(kb p) d -> p kb d", p=P)
            )
            nc.gpsimd.dma_start(qts[2][:], q_d[2])
            nc.gpsimd.dma_start(qts[3][:], q_d[3])

            # ---- software-pipelined attention stream ----
            # QK q-range start (in units of 128 cols) for block kb within
            # superblock sb: fully-masked supradiagonal regions trimmed.
            def q_lo(sb, kb):
                return max(0, kb - 4 * sb)

            pending = deque()
            pvmap = {}

            def drain():
                h, sb, pair, pt = pending.popleft()
                g = h // 2
                key = (h, sb)
                if key not in pvmap:
                    pvmap[key] = [
                        ppvp.tile(
                            [P, 2, D + 1], F32, tag="ppv", name=f"pv{h}_{sb}_{i}"
                        )
                        for i in range(2)
                    ]
                pvs = pvmap[key]
                for half in (0, 1):
                    kb = 2 * pair + half
                    for jj in range(4):
                        qb = 4 * sb + jj
                        if kb > qb:
                            continue
                        # start=True clears has_written bits for the WHOLE
                        # bank, so only the first region (even jj) of each
                        # bank may use it; the odd-jj group's first matmul
                        # relies on overwrite-where-bit-clear semantics.
                        nc.tensor.matmul(
                            pvs[jj // 2][:, jj % 2, :],
                            pt[:, half * SBW + jj * P : half * SBW + (jj + 1) * P],
                            vaugs[g][:, kb, :],
                            start=(kb == 0 and jj % 2 == 0),
                            stop=(kb == qb),
                            skip_group_check=(jj % 2 == 1),
                        )
                if pair == 2 * sb + 1:  # last pair of this superblock
                    for i in range(2):
                        ot = outp.tile(
                            [P, 2, D + 1], F32, tag="outs", name=f"o{h}_{sb}_{i}"
                        )
                        nc.vector.tensor_copy(ot[:], pvs[i][:])
                        nc.sync.dma_start(
                            o_d[h][:, 4 * sb + 2 * i : 4 * sb + 2 * i + 2, :], ot[:]
                        )
                    del pvmap[key]

            for h in range(NQH):
                g = h // 2
                qt, kt_g = qts[h], kts[g]
                for sb in range(NSB):
                    for pair in range(2 * sb + 2):
                        st = pstp.tile([P, 2 * SBW], F32, tag="pst", name="st")
                        for half in (0, 1):
                            kb = 2 * pair + half
                            lo = q_lo(sb, kb)
                            nc.tensor.matmul(
                                st[:, half * SBW + lo * P : (half + 1) * SBW],
                                kt_g[:, kb * P : (kb + 1) * P],
                                qt[:, sb * SBW + lo * P : (sb + 1) * SBW],
                                start=True,
                                stop=True,
                            )
                        pt = ptp.tile([P, 2 * SBW], BF16, tag="pt", name="pt")
                        # On the second diagonal pair, cols [0:256) of half 0
                        # are fully masked (kb=4sb+2 vs qb=4sb..): skip them.
                        elo = 2 * P if pair == 2 * sb + 1 else 0
                        nc.scalar.activation(
                            pt[:, elo : 2 * SBW],
                            st[:, elo : 2 * SBW],
                            mybir.ActivationFunctionType.Exp,
                            scale=SCALE,
                        )
                        if pair == 2 * sb:
                            mk = pair_masks[0]
                        elif pair == 2 * sb + 1:
                            mk = pair_masks[2]
                        else:
                            mk = None
                        if mk is not None:
                            nc.vector.tensor_tensor(
                                out=pt[:],
                                in0=pt[:],
                                in1=mk[:],
                                op=mybir.AluOpType.mult,
                            )
                        pending.append((h, sb, pair, pt))
                        if len(pending) > 1:
                            drain()
            while pending:
                drain()

    nc.finalize()
    return nc


def _get_module():
    if "nc" not in _CACHE:
        _CACHE["nc"] = build_module()
    return _CACHE["nc"]


def kernel(q, kv):
    global LAST_RESULTS
    q = np.asarray(q, dtype=np.float32)
    kv = np.asarray(kv, dtype=np.float32)

    nc = _get_module()
    bf = ml_dtypes.bfloat16
    in_maps = []
    for c in range(NCORES):
        b, j = divmod(c, 4)
        # q: [Sq, 4, D] -> [4, D, Sq]
        q_s = np.ascontiguousarray(
            q[b][:, 4 * j : 4 * j + 4, :].transpose(1, 2, 0).astype(bf)
        )
        # k: [Sk, 2, D] -> [2, D, Sk]
        k_s = np.ascontiguousarray(
            kv[b][:, 0, 2 * j : 2 * j + 2, :].transpose(1, 2, 0).astype(bf)
        )
        # v: [Sk, 2, D] -> [2, Sk, D]
        v_s = np.ascontiguousarray(
            kv[b][:, 1, 2 * j : 2 * j + 2, :].transpose(1, 0, 2).astype(bf)
        )
        in_maps.append({"q": q_s, "k": k_s, "v": v_s})

    trace = bool(int(os.environ.get("KERNEL_TRACE", "0")))
    kwargs = {}
    tdir = os.environ.get("KERNEL_TRACE_DIR")
    if tdir:
        kwargs["tmpdir"] = tdir
    if "warm" not in _CACHE:
        # Cold-start device executions intermittently read stale input
        # data (first execution after process start); run one warmup
        # execution and discard it so the measured run is warm.
        run_bass_kernel_spmd(nc, in_maps, core_ids=list(range(NCORES)), trace=False)
        _CACHE["warm"] = True
    res = run_bass_kernel_spmd(
        nc, in_maps, core_ids=list(range(NCORES)), trace=trace, **kwargs
    )
    LAST_RESULTS = res

    out = np.empty((B, SQ, H, D), np.float32)
    for c in range(NCORES):
        b, j = divmod(c, 4)
        o = res.results[c]["o"]  # [NQH, P, NQB, D+1]
        o = o.transpose(0, 2, 1, 3).reshape(NQH, SQ, D + 1)
        norm = o[..., :D] / o[..., D : D + 1]
        out[b, :, 4 * j : 4 * j + 4, :] = np.transpose(norm, (1, 0, 2))
    return out
